# revision 16
# baseline (speedup 1.0000x reference)
"""Trainium2 Bass kernel for nn_Block_74363063763569 (BEiT-style transformer block).

Data-parallel over batch across 8 NeuronCores (8 elems/core), zero collectives.
fp8e4m3 DoubleRow GEMMs; see build_nc docstring for the numerics scheme.
"""
import sys, json
sys.path.insert(0, "/opt/trn_rl_repo")
import numpy as np


def _legalize_waits(bir_bytes, max_waits=1):
    """This container's walrus rejects >1 sync wait per instruction; split
    extras into preceding single-wait EventSemaphore instructions."""
    j = json.loads(bir_bytes)
    for f in j["functions"]:
        for b in f["blocks"]:
            out = []
            for inst in b["instructions"]:
                si = inst.get("sync_info")
                waits = si.get("on_wait", []) if si else []
                if len(waits) > max_waits:
                    keep, extra = waits[:max_waits], waits[max_waits:]
                    for k, w in enumerate(extra):
                        out.append({"debug": inst.get("debug", 0), "engine": inst["engine"],
                                    "ins": [], "name": f"{inst['name']}_w{k}",
                                    "opcode": "EventSemaphore", "outs": [],
                                    "sync_info": {"on_update": [], "on_wait": [w]}})
                    si["on_wait"] = keep
                out.append(inst)
            b["instructions"] = out
    return json.dumps(j).encode()


import concourse.bass as bass
import concourse.tile as tile
import concourse.mybir as mybir
from concourse.masks import make_identity

FP32 = mybir.dt.float32
BF16 = mybir.dt.bfloat16
FP8 = mybir.dt.float8e4
DR = mybir.MatmulPerfMode.DoubleRow

B = 64
N = 197
C = 768
H = 12
D = 64
HID = 3072
NCORES = 8
BPC = B // NCORES
NPAIRS_FULL = BPC // 2
KG = C // 256      # 3 doublerow contraction groups over C
KG2 = HID // 256   # 12 groups over HID
LN_EPS = 1e-5

R = 256.0    # residual stream scale
SW = 64.0    # qkv/fc1 weight scale
SQ = 8.0     # q/k fp8 activation scale
SV = 8.0     # v fp8 activation scale (ones col = SV)

T_TILES = [(0, 128), (128, 69)]
T_PADS = [128, 72]           # padded token counts for fp8 stationary slices
EN = 200                     # padded per-elem token stride (4-aligned offsets)
TT = 2 * EN                  # packed token extent
C_CHUNKS = [(0, 512), (512, 256)]

AL = mybir.AluOpType
AF = mybir.ActivationFunctionType


def build_nc(npairs=NPAIRS_FULL):
    nb = 2 * npairs
    nc = bass.Bass()

    x_d = nc.dram_tensor("x", [nb, N, C], FP32, kind="ExternalInput")
    wqk_d = nc.dram_tensor("wqk", [KG, 128, 2, 1536], FP8, kind="ExternalInput")
    wv_d = nc.dram_tensor("wv", [KG, 128, 2, C], FP8, kind="ExternalInput")
    wvb_d = nc.dram_tensor("wvb", [1, 2, C], FP8, kind="ExternalInput")
    wp_d = nc.dram_tensor("wp", [KG, 128, 2, C], FP8, kind="ExternalInput")
    wf1_d = nc.dram_tensor("wf1", [KG, 128, 2, HID], FP8, kind="ExternalInput")
    wf2_d = nc.dram_tensor("wf2", [KG2, 128, 2, C], FP8, kind="ExternalInput")
    qkb_d = nc.dram_tensor("qkb", [128, 12], FP32, kind="ExternalInput")
    f1b_d = nc.dram_tensor("f1b", [128, 24], FP32, kind="ExternalInput")
    rpb0_d = nc.dram_tensor("rpb0", [128, H, EN], FP8, kind="ExternalInput")
    rpb1_d = nc.dram_tensor("rpb1", [72, H, EN], FP8, kind="ExternalInput")
    y_d = nc.dram_tensor("y", [nb, N, C], FP32, kind="ExternalOutput")

    with tile.TileContext(nc) as tc:
        with (
            tc.tile_pool(name="singles", bufs=1) as singles,
            tc.tile_pool(name="resid", bufs=2) as resid,
            tc.tile_pool(name="act", bufs=2) as act,      # per-pair fp8 activations
            tc.tile_pool(name="xn", bufs=3) as xnp,
            tc.tile_pool(name="expp", bufs=4) as expp,
            tc.tile_pool(name="small", bufs=3) as small,
            tc.tile_pool(name="ps_tr", bufs=2, space="PSUM") as ps_tr,
            tc.tile_pool(name="ps_mm", bufs=2, space="PSUM") as ps_mm,
            tc.tile_pool(name="ps_at", bufs=4, space="PSUM") as ps_at,
        ):
            # ---- persistent weights / constants ----
            wqk = [singles.tile([128, 2, 1536], FP8, tag=f"wqk{g}") for g in range(KG)]
            wv = [singles.tile([128, 2, C], FP8, tag=f"wv{g}") for g in range(KG)]
            wvb = singles.tile([1, 2, C], FP8, tag="wvb")
            wp = [singles.tile([128, 2, C], FP8, tag=f"wp{g}") for g in range(KG)]
            wf1 = [singles.tile([128, 2, HID], FP8, tag=f"wf1{g}") for g in range(KG)]
            wf2 = [singles.tile([128, 2, C], FP8, tag=f"wf2{g}") for g in range(KG2)]
            qkb = singles.tile([128, 12], FP32, tag="qkb")
            f1b = singles.tile([128, 24], FP32, tag="f1b")
            erpb = [singles.tile([128, H, N], BF16, tag="erpb0"),
                    singles.tile([69, H, N], BF16, tag="erpb1")]
            ident = singles.tile([128, 128], BF16, tag="ident")
            ones_x = singles.tile([1, 2, 128], FP8, tag="ones_x")
            ones_bf = singles.tile([1, 64], BF16, tag="ones_bf")
            eps_sb = singles.tile([128, 1], FP32, tag="eps")

            for g in range(KG):
                nc.sync.dma_start(wqk[g][:], wqk_d[g])
                nc.sync.dma_start(wv[g][:], wv_d[g])
                nc.sync.dma_start(wp[g][:], wp_d[g])
                nc.sync.dma_start(wf1[g][:], wf1_d[g])
            for g in range(KG2):
                nc.sync.dma_start(wf2[g][:], wf2_d[g])
            nc.sync.dma_start(wvb[:], wvb_d[:])
            nc.sync.dma_start(qkb[:], qkb_d[:])
            nc.sync.dma_start(f1b[:], f1b_d[:])
            nc.sync.dma_start(rpb8[0][:], rpb0_d[:])
            nc.sync.dma_start(rpb8[1][:], rpb1_d[:])
            make_identity(nc, ident[:])
            make_identity(nc, ident8[:])
            nc.vector.memset(ones_x[:], 1.0)
            nc.vector.memset(ones_bf[:], 1.0)
            nc.vector.memset(eps_sb[:], LN_EPS)

            def ln_transpose(x_tiles, tag):
                """LN over features + fp8 transpose into [128, 2, 2N] group tiles."""
                xT = [act.tile([128, 2, 2 * N], FP8, tag=f"{tag}T{g}") for g in range(KG)]
                for (e, j), xt in x_tiles.items():
                    toff, tcnt = T_TILES[j]
                    stats = small.tile([128, 3, 6], FP32, tag=f"st_{tag}")
                    mv = small.tile([128, 2], FP32, tag=f"mv_{tag}")
                    sd = small.tile([128, 1], FP32, tag=f"sd_{tag}")
                    rstd = small.tile([128, 1], FP32, tag=f"rs_{tag}")
                    for g3 in range(3):
                        nc.vector.bn_stats(stats[:tcnt, g3, :], xt[:tcnt, g3 * 256:(g3 + 1) * 256])
                    nc.vector.bn_aggr(mv[:tcnt], stats[:tcnt])
                    nc.scalar.activation(sd[:tcnt], mv[:tcnt, 1:2], AF.Ln, bias=eps_sb[:tcnt])
                    nc.scalar.activation(rstd[:tcnt], sd[:tcnt], AF.Exp, scale=-0.5)
                    xn8 = xnp.tile([128, C], BF16, tag="xn8")
                    nc.vector.tensor_scalar(
                        xn8[:tcnt, :], xt[:tcnt, :],
                        mv[:tcnt, 0:1], rstd[:tcnt, 0:1],
                        op0=AL.subtract, op1=AL.mult)
                    for cb in range(6):
                        pt = ps_tr.tile([128, 128], BF16, tag="ps_tr")
                        nc.tensor.transpose(
                            pt[:128, :tcnt],
                            xn8[:tcnt, cb * 128:(cb + 1) * 128],
                            ident[:tcnt, :tcnt])
                        dst = xT[cb // 2][:, cb % 2, e * EN + toff: e * EN + toff + tcnt]
                        if cb % 2 == 0:
                            nc.scalar.copy(dst, pt[:128, :tcnt])
                        else:
                            nc.vector.tensor_copy(dst, pt[:128, :tcnt])
                return xT

            for s in range(npairs):
                # ---------------- load x0 ----------------
                x0 = {}
                for e in range(2):
                    bidx = 2 * s + e
                    for j, (toff, tcnt) in enumerate(T_TILES):
                        t = resid.tile([128, C], FP32, tag=f"x0_{e}{j}")
                        nc.scalar.dma_start(t[:tcnt, :], x_d[bidx, toff:toff + tcnt, :])
                        x0[(e, j)] = t

                # ---------------- LN1 + fp8 transpose ----------------
                xnT = ln_transpose(x0, "ln1")

                # ---------------- q,k (weight-stationary DR) ----------------
                qkT = [act.tile([128, 2 * N], FP8, tag=f"qkT{b}") for b in range(12)]
                for b in range(12):
                    ps = ps_mm.tile([128, TT], FP32, tag="ps_mm")
                    for g in range(KG):
                        nc.tensor.matmul(
                            ps[:, :], wqk[g][:, :, b * 128:(b + 1) * 128],
                            xnT[g][:, :, :], start=(g == 0), stop=(g == KG - 1),
                            perf_mode=DR)
                    nc.vector.tensor_scalar(
                        qkT[b][:, :], ps[:, :], SQ / SW, qkb[:, b:b + 1],
                        op0=AL.mult, op1=AL.add)

                # ---------------- v (act-stationary DR) + vb row ----------------
                vt = {}
                for e in range(2):
                    v8 = act.tile([128, 2, H, 68], FP8, tag=f"vt{e}")
                    nc.gpsimd.memset(v8[:, :, :, :], 0.0)
                    nc.gpsimd.memset(v8[:, :, :, D:D + 1], SV)
                    vt[e] = v8
                for e in range(2):
                    for j, (toff, tcnt) in enumerate(T_TILES):
                        tp = T_PADS[j]
                        ts = e * EN + toff
                        for ci, (coff, csz) in enumerate(C_CHUNKS):
                            nh = csz // D
                            h0 = coff // D
                            ps = ps_mm.tile([128, 8, D], FP32, tag="ps_mm")
                            nc.tensor.matmul(
                                ps[:tp, :nh, :],
                                ones_x[:, :, :tp],
                                wvb[:, :, coff:coff + csz],
                                start=True, stop=False, perf_mode=DR)
                            for g in range(KG):
                                nc.tensor.matmul(
                                    ps[:tp, :nh, :],
                                    xnT[g][:, :, ts:ts + tp],
                                    wv[g][:, :, coff:coff + csz],
                                    start=False, stop=(g == KG - 1), perf_mode=DR)
                            nc.vector.tensor_scalar(
                                vt[e][:tcnt, j, h0:h0 + nh, 0:D],
                                ps[:tcnt, :nh, :], SV / SW, None, op0=AL.mult)

                # ---------------- attention ----------------
                aT = [act.tile([128, 2, 2 * N], FP8, tag=f"aT{g}") for g in range(KG)]
                for e in range(2):
                    for h in range(H):
                        qrow = 64 * (h % 2)
                        qt = qkT[h // 2]
                        kt = qkT[6 + h // 2]
                        et = expp.tile([128, 2, EN], FP8, tag="et")
                        nc.gpsimd.memset(et[64:, 1, :], 0.0)
                        nc.gpsimd.memset(et[:, :, N:], 0.0)
                        for j2, (tkoff, tkcnt) in enumerate(T_TILES):
                            tkp = T_PADS[j2]
                            L = ps_at.tile([128, EN], FP32, tag="ps_at")
                            nc.tensor.matmul(
                                L[:tkp, :EN],
                                kt[qrow:qrow + 64, e * EN + tkoff: e * EN + tkoff + tkp],
                                qt[qrow:qrow + 64, e * EN: e * EN + EN],
                                start=True, stop=False)
                            nc.tensor.matmul(
                                L[:tkp, :EN],
                                ident8[:tkp, :tkp],
                                rpb8[j2][:tkp, h, :],
                                start=False, stop=True)
                            nc.scalar.activation(et[:tkcnt, j2, :N], L[:tkcnt, :N],
                                                 AF.Exp, scale=1.0 / (SQ * SQ))
                        O = ps_at.tile([68, EN], FP32, tag="ps_at")
                        nc.tensor.matmul(
                            O[:68, :EN],
                            vt[e][:, :, h, :],
                            et[:, :, :], perf_mode=DR)
                        lden = small.tile([1, N], FP32, tag="lden")
                        rb = small.tile([1, N], BF16, tag="recip")
                        nc.scalar.activation(lden[:, :], O[D:D + 1, :N], AF.Ln)
                        nc.scalar.activation(rb[:, :], lden[:, :], AF.Exp, scale=-1.0)
                        Dn = ps_at.tile([64, N], FP32, tag="ps_at")
                        nc.tensor.matmul(Dn[:, :], ones_bf[0:1, :], rb[0:1, :])
                        Dsb = expp.tile([64, N], BF16, tag="Dsb")
                        nc.scalar.copy(Dsb[:, :], Dn[:, :])
                        nc.vector.tensor_tensor(
                            aT[h // 4][64 * (h % 2):64 * (h % 2) + 64, (h % 4) // 2,
                                       e * EN: e * EN + N],
                            O[0:D, :N], Dsb[:, :], op=AL.mult)

                # ---------------- proj + residual -> x1 ----------------
                x1 = {}
                for e in range(2):
                    for j, (toff, tcnt) in enumerate(T_TILES):
                        tp = T_PADS[j]
                        xt = resid.tile([128, C], FP32, tag=f"x1_{e}{j}")
                        ts = e * EN + toff
                        for ci, (coff, csz) in enumerate(C_CHUNKS):
                            ps = ps_mm.tile([128, 512], FP32, tag="ps_mm")
                            for g in range(KG):
                                nc.tensor.matmul(
                                    ps[:tp, :csz],
                                    aT[g][:, :, ts:ts + tp],
                                    wp[g][:, :, coff:coff + csz],
                                    start=(g == 0), stop=(g == KG - 1), perf_mode=DR)
                            nc.vector.tensor_tensor(
                                xt[:tcnt, coff:coff + csz],
                                ps[:tcnt, :csz],
                                x0[(e, j)][:tcnt, coff:coff + csz], op=AL.add)
                        x1[(e, j)] = xt

                # ---------------- LN2 + fp8 transpose ----------------
                hnT = ln_transpose(x1, "ln2")

                # ---------------- fc1 + gelu -> hT ----------------
                hT = [act.tile([128, 2, 2 * N], FP8, tag=f"hT{g}") for g in range(KG2)]
                for ob in range(24):
                    ps = ps_mm.tile([128, TT], FP32, tag="ps_mm")
                    for g in range(KG):
                        nc.tensor.matmul(
                            ps[:, :], wf1[g][:, :, ob * 128:(ob + 1) * 128],
                            hnT[g][:, :, :], start=(g == 0), stop=(g == KG - 1),
                            perf_mode=DR)
                    nc.scalar.activation(
                        hT[ob // 2][:, ob % 2, :], ps[:, :], AF.Gelu,
                        scale=1.0 / SW, bias=f1b[:, ob:ob + 1])

                # ---------------- fc2 + residual -> y ----------------
                for e in range(2):
                    bidx = 2 * s + e
                    for j, (toff, tcnt) in enumerate(T_TILES):
                        tp = T_PADS[j]
                        ot = resid.tile([128, C], FP32, tag=f"x0_{e}{j}", name=f"out_{e}{j}", bufs=3)
                        ts = e * EN + toff
                        for ci, (coff, csz) in enumerate(C_CHUNKS):
                            ps = ps_mm.tile([128, 512], FP32, tag="ps_mm")
                            for g in range(KG2):
                                nc.tensor.matmul(
                                    ps[:tp, :csz],
                                    hT[g][:, :, ts:ts + tp],
                                    wf2[g][:, :, coff:coff + csz],
                                    start=(g == 0), stop=(g == KG2 - 1), perf_mode=DR)
                            nc.vector.tensor_tensor(
                                ot[:tcnt, coff:coff + csz],
                                ps[:tcnt, :csz],
                                x1[(e, j)][:tcnt, coff:coff + csz], op=AL.add)
                        nc.gpsimd.dma_start(y_d[bidx, toff:toff + tcnt, :], ot[:tcnt, :])

    return nc


def fold_weights(inputs):
    """Host-side folding into fp8 DoubleRow layouts. Returns per-core dict."""
    import ml_dtypes
    f8 = ml_dtypes.float8_e4m3
    bf = ml_dtypes.bfloat16
    f32 = np.float32
    g = {k: np.asarray(v) for k, v in inputs.items()}
    n1w, n1b = g["n1_w"].astype(f32), g["n1_b"].astype(f32)
    n2w, n2b = g["n2_w"].astype(f32), g["n2_b"].astype(f32)
    g1, g2 = g["gamma1"].astype(f32), g["gamma2"].astype(f32)
    qkv_w = g["qkv_w"].astype(f32)
    q_bias, v_bias = g["q_bias"].astype(f32), g["v_bias"].astype(f32)
    proj_w, proj_b = g["proj_w"].astype(f32), g["proj_b"].astype(f32)
    fc1_w, fc1_b = g["fc1_w"].astype(f32), g["fc1_b"].astype(f32)
    fc2_w, fc2_b = g["fc2_w"].astype(f32), g["fc2_b"].astype(f32)
    assert np.all(proj_b == 0), "kernel assumes proj_b == 0"

    qkv_bias = np.concatenate([q_bias, np.zeros_like(q_bias), v_bias])
    Wq = qkv_w * n1w[None, :]                       # LN affine fold
    bq = qkv_bias + qkv_w @ n1b
    scale = D ** -0.5
    Wq[:C] *= scale
    bq[:C] *= scale

    def dr_pack(wT, ngroups):
        # wT: [in_features, out] -> [ngroups, 128, 2, out]
        nin = wT.shape[0]
        assert nin == ngroups * 256
        return np.ascontiguousarray(
            wT.reshape(ngroups, 2, 128, -1).transpose(0, 2, 1, 3))

    wqk = dr_pack((SW * Wq[:2 * C].T), KG).astype(f8)
    wv = dr_pack((SW * Wq[2 * C:].T), KG).astype(f8)
    wvb = np.zeros((1, 2, C), np.float32)
    wvb[0, 0, :] = SW * bq[2 * C:]
    Pw = (g1[:, None] * proj_w)
    wp = dr_pack((R * Pw.T), KG).astype(f8)
    F1 = fc1_w * n2w[None, :]
    f1b_full = fc1_b + fc1_w @ n2b
    wf1 = dr_pack((SW * F1.T), KG).astype(f8)
    F2 = g2[:, None] * fc2_w
    wf2 = dr_pack((R * F2.T), KG2).astype(f8)

    qkb = (SQ * bq[:2 * C]).reshape(12, 128).T.copy()
    f1b = f1b_full.reshape(24, 128).T.copy()

    table = g["rel_bias_table"].astype(f32)
    idx = np.asarray(g["rel_index"]).reshape(-1)
    rpb = table[idx].reshape(N, N, H).transpose(2, 0, 1)   # [h, tq, tk]
    rpbT = rpb.transpose(0, 2, 1)                          # [h, tk, tq]
    rpb0 = np.zeros((128, H, 200), np.float32)
    rpb1 = np.zeros((72, H, 200), np.float32)
    rpb0[:, :, :N] = (SQ * SQ) * rpbT[:, :128, :].transpose(1, 0, 2)
    rpb1[:69, :, :N] = (SQ * SQ) * rpbT[:, 128:, :].transpose(1, 0, 2)

    return {
        "wqk": wqk, "wv": wv, "wvb": wvb.astype(f8), "wp": wp,
        "wf1": wf1, "wf2": wf2,
        "qkb": np.ascontiguousarray(qkb), "f1b": np.ascontiguousarray(f1b),
        "rpb0": rpb0.astype(f8), "rpb1": rpb1.astype(f8),
    }, (g2 * fc2_b).astype(f32)


_CACHE = {}


def _get_nc():
    if "nc" not in _CACHE:
        nc = build_nc()
        patched = _legalize_waits(nc.to_json_bytes())
        nc.to_json_bytes = lambda: patched
        _CACHE["nc"] = nc
    return _CACHE["nc"]


def kernel(**inputs):
    from concourse.bass_utils import run_bass_kernel_spmd
    nc = _get_nc()
    folded, f2b_host = fold_weights(inputs)
    x = np.ascontiguousarray(np.asarray(inputs["x"], dtype=np.float32))
    assert x.shape == (B, N, C), x.shape
    xs = R * x
    in_maps = []
    for c in range(NCORES):
        m = dict(folded)
        m["x"] = np.ascontiguousarray(xs[c * BPC:(c + 1) * BPC])
        in_maps.append(m)
    res = run_bass_kernel_spmd(nc, in_maps, core_ids=list(range(NCORES)))
    out = np.concatenate([res.results[c]["y"] for c in range(NCORES)], axis=0)
    return (out * (1.0 / R) + f2b_host).astype(np.float32)


# revision 17
# speedup vs baseline: 1.2781x; 1.2781x over previous
"""Trainium2 Bass kernel for nn_Block_74363063763569 (BEiT-style transformer block).

Data-parallel over batch across 8 NeuronCores (8 elems/core), zero collectives.
fp8e4m3 DoubleRow GEMMs; see build_nc docstring for the numerics scheme.
"""
import sys, json
sys.path.insert(0, "/opt/trn_rl_repo")
import numpy as np


def _legalize_waits(bir_bytes, max_waits=1):
    """This container's walrus rejects >1 sync wait per instruction; split
    extras into preceding single-wait EventSemaphore instructions."""
    j = json.loads(bir_bytes)
    for f in j["functions"]:
        for b in f["blocks"]:
            out = []
            for inst in b["instructions"]:
                si = inst.get("sync_info")
                waits = si.get("on_wait", []) if si else []
                if len(waits) > max_waits:
                    keep, extra = waits[:max_waits], waits[max_waits:]
                    for k, w in enumerate(extra):
                        out.append({"debug": inst.get("debug", 0), "engine": inst["engine"],
                                    "ins": [], "name": f"{inst['name']}_w{k}",
                                    "opcode": "EventSemaphore", "outs": [],
                                    "sync_info": {"on_update": [], "on_wait": [w]}})
                    si["on_wait"] = keep
                out.append(inst)
            b["instructions"] = out
    return json.dumps(j).encode()


import concourse.bass as bass
import concourse.tile as tile
import concourse.mybir as mybir
from concourse.masks import make_identity

FP32 = mybir.dt.float32
BF16 = mybir.dt.bfloat16
FP8 = mybir.dt.float8e4
DR = mybir.MatmulPerfMode.DoubleRow

B = 64
N = 197
C = 768
H = 12
D = 64
HID = 3072
NCORES = 8
BPC = B // NCORES
NPAIRS_FULL = BPC // 2
KG = C // 256      # 3 doublerow contraction groups over C
KG2 = HID // 256   # 12 groups over HID
LN_EPS = 1e-5

R = 256.0    # residual stream scale
SW = 64.0    # qkv/fc1 weight scale
SQ = 8.0     # q/k fp8 activation scale
SV = 8.0     # v fp8 activation scale (ones col = SV)

T_TILES = [(0, 128), (128, 69)]
T_PADS = [128, 72]           # padded token counts for fp8 stationary slices
EN = 200                     # padded per-elem token stride (4-aligned offsets)
TT = 2 * EN                  # packed token extent
C_CHUNKS = [(0, 512), (512, 256)]

AL = mybir.AluOpType
AF = mybir.ActivationFunctionType


def build_nc(npairs=NPAIRS_FULL):
    nb = 2 * npairs
    nc = bass.Bass()

    x_d = nc.dram_tensor("x", [nb, N, C], FP32, kind="ExternalInput")
    wqk_d = nc.dram_tensor("wqk", [KG, 128, 2, 1536], FP8, kind="ExternalInput")
    wv_d = nc.dram_tensor("wv", [KG, 128, 2, C], FP8, kind="ExternalInput")
    wvb_d = nc.dram_tensor("wvb", [1, 2, C], FP8, kind="ExternalInput")
    wp_d = nc.dram_tensor("wp", [KG, 128, 2, C], FP8, kind="ExternalInput")
    wf1_d = nc.dram_tensor("wf1", [KG, 128, 2, HID], FP8, kind="ExternalInput")
    wf2_d = nc.dram_tensor("wf2", [KG2, 128, 2, C], FP8, kind="ExternalInput")
    qkb_d = nc.dram_tensor("qkb", [128, 12], FP32, kind="ExternalInput")
    f1b_d = nc.dram_tensor("f1b", [128, 24], FP32, kind="ExternalInput")
    rpb0_d = nc.dram_tensor("rpb0", [128, H, EN], FP8, kind="ExternalInput")
    rpb1_d = nc.dram_tensor("rpb1", [72, H, EN], FP8, kind="ExternalInput")
    y_d = nc.dram_tensor("y", [nb, N, C], FP32, kind="ExternalOutput")

    with tile.TileContext(nc) as tc:
        with (
            tc.tile_pool(name="singles", bufs=1) as singles,
            tc.tile_pool(name="resid", bufs=2) as resid,
            tc.tile_pool(name="act", bufs=2) as act,      # per-pair fp8 activations
            tc.tile_pool(name="xn", bufs=3) as xnp,
            tc.tile_pool(name="expp", bufs=3) as expp,
            tc.tile_pool(name="small", bufs=3) as small,
            tc.tile_pool(name="ps_tr", bufs=2, space="PSUM") as ps_tr,
            tc.tile_pool(name="ps_mm", bufs=2, space="PSUM") as ps_mm,
            tc.tile_pool(name="ps_at", bufs=4, space="PSUM") as ps_at,
        ):
            # ---- persistent weights / constants ----
            wqk = [singles.tile([128, 2, 1536], FP8, tag=f"wqk{g}") for g in range(KG)]
            wv = [singles.tile([128, 2, C], FP8, tag=f"wv{g}") for g in range(KG)]
            wvb = singles.tile([1, 2, C], FP8, tag="wvb")
            wp = [singles.tile([128, 2, C], FP8, tag=f"wp{g}") for g in range(KG)]
            wf1 = [singles.tile([128, 2, HID], FP8, tag=f"wf1{g}") for g in range(KG)]
            wf2 = [singles.tile([128, 2, C], FP8, tag=f"wf2{g}") for g in range(KG2)]
            qkb = singles.tile([128, 12], FP32, tag="qkb")
            f1b = singles.tile([128, 24], FP32, tag="f1b")
            erpb = [singles.tile([128, H, N], BF16, tag="erpb0"),
                    singles.tile([69, H, N], BF16, tag="erpb1")]
            ident = singles.tile([128, 128], BF16, tag="ident")
            ones_x = singles.tile([1, 2, 128], FP8, tag="ones_x")
            ones_bf = singles.tile([1, 64], BF16, tag="ones_bf")
            eps_sb = singles.tile([128, 1], FP32, tag="eps")

            for g in range(KG):
                nc.sync.dma_start(wqk[g][:], wqk_d[g])
                nc.sync.dma_start(wv[g][:], wv_d[g])
                nc.sync.dma_start(wp[g][:], wp_d[g])
                nc.sync.dma_start(wf1[g][:], wf1_d[g])
            for g in range(KG2):
                nc.sync.dma_start(wf2[g][:], wf2_d[g])
            nc.sync.dma_start(wvb[:], wvb_d[:])
            nc.sync.dma_start(qkb[:], qkb_d[:])
            nc.sync.dma_start(f1b[:], f1b_d[:])
            nc.sync.dma_start(rpb8[0][:], rpb0_d[:])
            nc.sync.dma_start(rpb8[1][:], rpb1_d[:])
            make_identity(nc, ident[:])
            make_identity(nc, ident8[:])
            nc.vector.memset(ones_x[:], 1.0)
            nc.vector.memset(ones_bf[:], 1.0)
            nc.vector.memset(eps_sb[:], LN_EPS)

            def ln_transpose(x_tiles, tag):
                """LN over features + fp8 transpose into [128, 2, 2N] group tiles."""
                xT = [act.tile([128, 2, 2 * N], FP8, tag=f"{tag}T{g}") for g in range(KG)]
                for (e, j), xt in x_tiles.items():
                    toff, tcnt = T_TILES[j]
                    stats = small.tile([128, 3, 6], FP32, tag=f"st_{tag}")
                    mv = small.tile([128, 2], FP32, tag=f"mv_{tag}")
                    sd = small.tile([128, 1], FP32, tag=f"sd_{tag}")
                    rstd = small.tile([128, 1], FP32, tag=f"rs_{tag}")
                    for g3 in range(3):
                        nc.vector.bn_stats(stats[:tcnt, g3, :], xt[:tcnt, g3 * 256:(g3 + 1) * 256])
                    nc.vector.bn_aggr(mv[:tcnt], stats[:tcnt])
                    nc.scalar.activation(sd[:tcnt], mv[:tcnt, 1:2], AF.Ln, bias=eps_sb[:tcnt])
                    nc.scalar.activation(rstd[:tcnt], sd[:tcnt], AF.Exp, scale=-0.5)
                    xn8 = xnp.tile([128, C], BF16, tag="xn8")
                    nc.vector.tensor_scalar(
                        xn8[:tcnt, :], xt[:tcnt, :],
                        mv[:tcnt, 0:1], rstd[:tcnt, 0:1],
                        op0=AL.subtract, op1=AL.mult)
                    for cb in range(6):
                        pt = ps_tr.tile([128, 128], BF16, tag="ps_tr")
                        nc.tensor.transpose(
                            pt[:128, :tcnt],
                            xn8[:tcnt, cb * 128:(cb + 1) * 128],
                            ident[:tcnt, :tcnt])
                        dst = xT[cb // 2][:, cb % 2, e * EN + toff: e * EN + toff + tcnt]
                        if cb % 2 == 0:
                            nc.scalar.copy(dst, pt[:128, :tcnt])
                        else:
                            nc.vector.tensor_copy(dst, pt[:128, :tcnt])
                return xT

            for s in range(npairs):
                # ---------------- load x0 ----------------
                x0 = {}
                for e in range(2):
                    bidx = 2 * s + e
                    for j, (toff, tcnt) in enumerate(T_TILES):
                        t = resid.tile([128, C], FP32, tag=f"x0_{e}{j}")
                        nc.scalar.dma_start(t[:tcnt, :], x_d[bidx, toff:toff + tcnt, :])
                        x0[(e, j)] = t

                # ---------------- LN1 + fp8 transpose ----------------
                xnT = ln_transpose(x0, "ln1")

                # ---------------- q,k (weight-stationary DR) ----------------
                qkT = [act.tile([128, 2 * N], FP8, tag=f"qkT{b}") for b in range(12)]
                for b in range(12):
                    ps = ps_mm.tile([128, TT], FP32, tag="ps_mm")
                    for g in range(KG):
                        nc.tensor.matmul(
                            ps[:, :], wqk[g][:, :, b * 128:(b + 1) * 128],
                            xnT[g][:, :, :], start=(g == 0), stop=(g == KG - 1),
                            perf_mode=DR)
                    nc.vector.tensor_scalar(
                        qkT[b][:, :], ps[:, :], SQ / SW, qkb[:, b:b + 1],
                        op0=AL.mult, op1=AL.add)

                # ---------------- v (act-stationary DR) + vb row ----------------
                vt = {}
                for e in range(2):
                    v8 = act.tile([128, 2, H, 68], FP8, tag=f"vt{e}")
                    nc.gpsimd.memset(v8[:, :, :, :], 0.0)
                    nc.gpsimd.memset(v8[:, :, :, D:D + 1], SV)
                    vt[e] = v8
                for e in range(2):
                    for j, (toff, tcnt) in enumerate(T_TILES):
                        tp = T_PADS[j]
                        ts = e * EN + toff
                        for ci, (coff, csz) in enumerate(C_CHUNKS):
                            nh = csz // D
                            h0 = coff // D
                            ps = ps_mm.tile([128, 8, D], FP32, tag="ps_mm")
                            nc.tensor.matmul(
                                ps[:tp, :nh, :],
                                ones_x[:, :, :tp],
                                wvb[:, :, coff:coff + csz],
                                start=True, stop=False, perf_mode=DR)
                            for g in range(KG):
                                nc.tensor.matmul(
                                    ps[:tp, :nh, :],
                                    xnT[g][:, :, ts:ts + tp],
                                    wv[g][:, :, coff:coff + csz],
                                    start=False, stop=(g == KG - 1), perf_mode=DR)
                            nc.vector.tensor_scalar(
                                vt[e][:tcnt, j, h0:h0 + nh, 0:D],
                                ps[:tcnt, :nh, :], SV / SW, None, op0=AL.mult)

                # ---------------- attention ----------------
                aT = [act.tile([128, 2, 2 * N], FP8, tag=f"aT{g}") for g in range(KG)]
                for e in range(2):
                    for h in range(H):
                        qrow = 64 * (h % 2)
                        qt = qkT[h // 2]
                        kt = qkT[6 + h // 2]
                        et = expp.tile([128, 2, EN], FP8, tag="et")
                        nc.gpsimd.memset(et[64:, 1, :], 0.0)
                        nc.gpsimd.memset(et[:, :, N:], 0.0)
                        for j2, (tkoff, tkcnt) in enumerate(T_TILES):
                            tkp = T_PADS[j2]
                            L = ps_at.tile([128, EN], FP32, tag="ps_at")
                            nc.tensor.matmul(
                                L[:tkp, :EN],
                                kt[qrow:qrow + 64, e * EN + tkoff: e * EN + tkoff + tkp],
                                qt[qrow:qrow + 64, e * EN: e * EN + EN],
                                start=True, stop=False)
                            nc.tensor.matmul(
                                L[:tkp, :EN],
                                ident8[:tkp, :tkp],
                                rpb8[j2][:tkp, h, :],
                                start=False, stop=True)
                            nc.scalar.activation(et[:tkcnt, j2, :N], L[:tkcnt, :N],
                                                 AF.Exp, scale=1.0 / (SQ * SQ))
                        O = ps_at.tile([68, EN], FP32, tag="ps_at")
                        nc.tensor.matmul(
                            O[:68, :EN],
                            vt[e][:, :, h, :],
                            et[:, :, :], perf_mode=DR)
                        lden = small.tile([1, N], FP32, tag="lden")
                        rb = small.tile([1, N], BF16, tag="recip")
                        nc.scalar.activation(lden[:, :], O[D:D + 1, :N], AF.Ln)
                        nc.scalar.activation(rb[:, :], lden[:, :], AF.Exp, scale=-1.0)
                        Dn = ps_at.tile([64, N], FP32, tag="ps_at")
                        nc.tensor.matmul(Dn[:, :], ones_bf[0:1, :], rb[0:1, :])
                        Dsb = expp.tile([64, N], BF16, tag="Dsb")
                        nc.scalar.copy(Dsb[:, :], Dn[:, :])
                        nc.vector.tensor_tensor(
                            aT[h // 4][64 * (h % 2):64 * (h % 2) + 64, (h % 4) // 2,
                                       e * EN: e * EN + N],
                            O[0:D, :N], Dsb[:, :], op=AL.mult)

                # ---------------- proj + residual -> x1 ----------------
                x1 = {}
                for e in range(2):
                    for j, (toff, tcnt) in enumerate(T_TILES):
                        tp = T_PADS[j]
                        xt = resid.tile([128, C], FP32, tag=f"x1_{e}{j}")
                        ts = e * EN + toff
                        for ci, (coff, csz) in enumerate(C_CHUNKS):
                            ps = ps_mm.tile([128, 512], FP32, tag="ps_mm")
                            for g in range(KG):
                                nc.tensor.matmul(
                                    ps[:tp, :csz],
                                    aT[g][:, :, ts:ts + tp],
                                    wp[g][:, :, coff:coff + csz],
                                    start=(g == 0), stop=(g == KG - 1), perf_mode=DR)
                            nc.vector.tensor_tensor(
                                xt[:tcnt, coff:coff + csz],
                                ps[:tcnt, :csz],
                                x0[(e, j)][:tcnt, coff:coff + csz], op=AL.add)
                        x1[(e, j)] = xt

                # ---------------- LN2 + fp8 transpose ----------------
                hnT = ln_transpose(x1, "ln2")

                # ---------------- fc1 + gelu -> hT ----------------
                hT = [act.tile([128, 2, 2 * N], FP8, tag=f"hT{g}") for g in range(KG2)]
                for ob in range(24):
                    ps = ps_mm.tile([128, TT], FP32, tag="ps_mm")
                    for g in range(KG):
                        nc.tensor.matmul(
                            ps[:, :], wf1[g][:, :, ob * 128:(ob + 1) * 128],
                            hnT[g][:, :, :], start=(g == 0), stop=(g == KG - 1),
                            perf_mode=DR)
                    nc.scalar.activation(
                        hT[ob // 2][:, ob % 2, :], ps[:, :], AF.Gelu,
                        scale=1.0 / SW, bias=f1b[:, ob:ob + 1])

                # ---------------- fc2 + residual -> y ----------------
                for e in range(2):
                    bidx = 2 * s + e
                    for j, (toff, tcnt) in enumerate(T_TILES):
                        tp = T_PADS[j]
                        ot = resid.tile([128, C], FP32, tag=f"x0_{e}{j}", name=f"out_{e}{j}", bufs=3)
                        ts = e * EN + toff
                        for ci, (coff, csz) in enumerate(C_CHUNKS):
                            ps = ps_mm.tile([128, 512], FP32, tag="ps_mm")
                            for g in range(KG2):
                                nc.tensor.matmul(
                                    ps[:tp, :csz],
                                    hT[g][:, :, ts:ts + tp],
                                    wf2[g][:, :, coff:coff + csz],
                                    start=(g == 0), stop=(g == KG2 - 1), perf_mode=DR)
                            nc.vector.tensor_tensor(
                                ot[:tcnt, coff:coff + csz],
                                ps[:tcnt, :csz],
                                x1[(e, j)][:tcnt, coff:coff + csz], op=AL.add)
                        nc.gpsimd.dma_start(y_d[bidx, toff:toff + tcnt, :], ot[:tcnt, :])

    return nc


def fold_weights(inputs):
    """Host-side folding into fp8 DoubleRow layouts. Returns per-core dict."""
    import ml_dtypes
    f8 = ml_dtypes.float8_e4m3
    bf = ml_dtypes.bfloat16
    f32 = np.float32
    g = {k: np.asarray(v) for k, v in inputs.items()}
    n1w, n1b = g["n1_w"].astype(f32), g["n1_b"].astype(f32)
    n2w, n2b = g["n2_w"].astype(f32), g["n2_b"].astype(f32)
    g1, g2 = g["gamma1"].astype(f32), g["gamma2"].astype(f32)
    qkv_w = g["qkv_w"].astype(f32)
    q_bias, v_bias = g["q_bias"].astype(f32), g["v_bias"].astype(f32)
    proj_w, proj_b = g["proj_w"].astype(f32), g["proj_b"].astype(f32)
    fc1_w, fc1_b = g["fc1_w"].astype(f32), g["fc1_b"].astype(f32)
    fc2_w, fc2_b = g["fc2_w"].astype(f32), g["fc2_b"].astype(f32)
    assert np.all(proj_b == 0), "kernel assumes proj_b == 0"

    qkv_bias = np.concatenate([q_bias, np.zeros_like(q_bias), v_bias])
    Wq = qkv_w * n1w[None, :]                       # LN affine fold
    bq = qkv_bias + qkv_w @ n1b
    scale = D ** -0.5
    Wq[:C] *= scale
    bq[:C] *= scale

    def dr_pack(wT, ngroups):
        # wT: [in_features, out] -> [ngroups, 128, 2, out]
        nin = wT.shape[0]
        assert nin == ngroups * 256
        return np.ascontiguousarray(
            wT.reshape(ngroups, 2, 128, -1).transpose(0, 2, 1, 3))

    wqk = dr_pack((SW * Wq[:2 * C].T), KG).astype(f8)
    wv = dr_pack((SW * Wq[2 * C:].T), KG).astype(f8)
    wvb = np.zeros((1, 2, C), np.float32)
    wvb[0, 0, :] = SW * bq[2 * C:]
    Pw = (g1[:, None] * proj_w)
    wp = dr_pack((R * Pw.T), KG).astype(f8)
    F1 = fc1_w * n2w[None, :]
    f1b_full = fc1_b + fc1_w @ n2b
    wf1 = dr_pack((SW * F1.T), KG).astype(f8)
    F2 = g2[:, None] * fc2_w
    wf2 = dr_pack((R * F2.T), KG2).astype(f8)

    qkb = (SQ * bq[:2 * C]).reshape(12, 128).T.copy()
    f1b = f1b_full.reshape(24, 128).T.copy()

    table = g["rel_bias_table"].astype(f32)
    idx = np.asarray(g["rel_index"]).reshape(-1)
    rpb = table[idx].reshape(N, N, H).transpose(2, 0, 1)   # [h, tq, tk]
    rpbT = rpb.transpose(0, 2, 1)                          # [h, tk, tq]
    rpb0 = np.zeros((128, H, 200), np.float32)
    rpb1 = np.zeros((72, H, 200), np.float32)
    rpb0[:, :, :N] = (SQ * SQ) * rpbT[:, :128, :].transpose(1, 0, 2)
    rpb1[:69, :, :N] = (SQ * SQ) * rpbT[:, 128:, :].transpose(1, 0, 2)

    return {
        "wqk": wqk, "wv": wv, "wvb": wvb.astype(f8), "wp": wp,
        "wf1": wf1, "wf2": wf2,
        "qkb": np.ascontiguousarray(qkb), "f1b": np.ascontiguousarray(f1b),
        "rpb0": rpb0.astype(f8), "rpb1": rpb1.astype(f8),
    }, (g2 * fc2_b).astype(f32)


_CACHE = {}


def _get_nc():
    if "nc" not in _CACHE:
        nc = build_nc()
        patched = _legalize_waits(nc.to_json_bytes())
        nc.to_json_bytes = lambda: patched
        _CACHE["nc"] = nc
    return _CACHE["nc"]


def kernel(**inputs):
    from concourse.bass_utils import run_bass_kernel_spmd
    nc = _get_nc()
    folded, f2b_host = fold_weights(inputs)
    x = np.ascontiguousarray(np.asarray(inputs["x"], dtype=np.float32))
    assert x.shape == (B, N, C), x.shape
    xs = R * x
    in_maps = []
    for c in range(NCORES):
        m = dict(folded)
        m["x"] = np.ascontiguousarray(xs[c * BPC:(c + 1) * BPC])
        in_maps.append(m)
    res = run_bass_kernel_spmd(nc, in_maps, core_ids=list(range(NCORES)))
    out = np.concatenate([res.results[c]["y"] for c in range(NCORES)], axis=0)
    return (out * (1.0 / R) + f2b_host).astype(np.float32)
